# revision 4
# baseline (speedup 1.0000x reference)
"""Trainium2 Bass kernel for CustomMultiHeadAttention (sparse_attention).

Data-parallel over the positive-sample axis bs (8 cores, one y-batch per
core, 8 aligned x-batches per core). Projection weights replicated.

Per core c (handles y[c] and x[c*8:(c+1)*8]):
  - QT/KT (d-on-partitions) and V (rows-on-partitions) via PE matmuls from
    host-pretransposed xT/yT and WT.
  - scores = QT.T @ KT with the column-mask fold (m_j-1)*1e10 added as a
    K=1 matmul accumulation; row mask applied as tensor_scalar
    (psum*m_i + (m_i-1)*1e10) which also lands masked rows at exactly
    -1e10 in f32 (|S| < 512 = ulp(1e10)/2).
  - softmax: reduce_max(negate) -> ACT exp(bias=-max, accum_out=rowsum);
    fully-masked rows come out uniform like the reference.
  - ctx: E transposed via PE (identity), ctx = ET.T-slices @ V, normalized
    by 1/rowsum on PSUM readout.
  - probs_xy = E * (1/rowsum), written for all 8 x-batches vs this core's y.

mask_xy / y_len are pure functions of the masks -> computed on host.
"""

import os

import numpy as np

import concourse.bass as bass
import concourse.mybir as mybir
import concourse.tile as tile
from concourse import bacc
from concourse.bass_utils import run_bass_kernel_spmd
from concourse.masks import make_identity

B, BS, M, D = 64, 8, 512, 256
NEG = B // BS  # 8 x-batches per core
P = 128
NT = M // P  # 4 row tiles per batch
KT = D // P  # 2 contraction tiles
F32 = mybir.dt.float32
NBATCH = NEG + 1  # 8 x-batches + 1 y-batch per core
ALU = mybir.AluOpType
AF = mybir.ActivationFunctionType

_CACHE = {}
LAST_RESULT = None  # test harness can inspect exec_time_ns


def build_kernel():
    nc = bacc.Bacc(
        "TRN2",
        target_bir_lowering=False,
        debug=False,
        enable_asserts=False,
        num_devices=BS,
    )

    xT = nc.dram_tensor("xT", [NEG, KT, P, M], F32, kind="ExternalInput")
    yT = nc.dram_tensor("yT", [KT, P, M], F32, kind="ExternalInput")
    wqT = nc.dram_tensor("wqT", [KT, P, D], F32, kind="ExternalInput")
    wkT = nc.dram_tensor("wkT", [KT, P, D], F32, kind="ExternalInput")
    wvT = nc.dram_tensor("wvT", [KT, P, D], F32, kind="ExternalInput")
    # Row-mask multiplier / additive fill, one column per (batch, row-tile).
    rowm_d = nc.dram_tensor("rowm", [P, NBATCH * NT], F32, kind="ExternalInput")
    rowa_d = nc.dram_tensor("rowa", [P, NBATCH * NT], F32, kind="ExternalInput")
    # Column-mask fold (m-1)*1e10 per batch (row 8 = y).
    colf_d = nc.dram_tensor("colf", [1, NBATCH * M], F32, kind="ExternalInput")

    scores_x = nc.dram_tensor("scores_x", [NEG, M, M], F32, kind="ExternalOutput")
    probs_xy = nc.dram_tensor("probs_xy", [NEG, M, M], F32, kind="ExternalOutput")
    scores_y = nc.dram_tensor("scores_y", [M, M], F32, kind="ExternalOutput")
    ctx_x = nc.dram_tensor("ctx_x", [NEG, M, D], F32, kind="ExternalOutput")
    ctx_y = nc.dram_tensor("ctx_y", [M, D], F32, kind="ExternalOutput")

    with tile.TileContext(nc) as tc:
        with (
            tc.tile_pool(name="const", bufs=1) as const,
            tc.tile_pool(name="persist", bufs=1) as persist,
            tc.tile_pool(name="work", bufs=2) as work,
            tc.tile_pool(name="sout", bufs=4) as sout,
            tc.tile_pool(name="small", bufs=12) as small,
            tc.tile_pool(name="psA", bufs=4, space="PSUM") as psA,
            tc.tile_pool(name="psB", bufs=2, space="PSUM") as psB,
            tc.tile_pool(name="psC", bufs=2, space="PSUM") as psC,
        ):
            # ---- constants ----
            wq_sb = const.tile([P, KT, D], F32, tag="wq")
            wk_sb = const.tile([P, KT, D], F32, tag="wk")
            wv_sb = const.tile([P, KT, D], F32, tag="wv")
            nc.sync.dma_start(wq_sb[:], wqT.rearrange("k p d -> p k d"))
            nc.sync.dma_start(wk_sb[:], wkT.rearrange("k p d -> p k d"))
            nc.sync.dma_start(wv_sb[:], wvT.rearrange("k p d -> p k d"))
            rowm_sb = const.tile([P, NBATCH * NT], F32, tag="rowm")
            rowa_sb = const.tile([P, NBATCH * NT], F32, tag="rowa")
            colf_sb = const.tile([1, NBATCH * M], F32, tag="colf")
            nc.sync.dma_start(rowm_sb[:], rowm_d[:])
            nc.sync.dma_start(rowa_sb[:], rowa_d[:])
            nc.sync.dma_start(colf_sb[:], colf_d[:])
            ident = const.tile([P, P], F32, tag="ident")
            make_identity(nc, ident[:])
            ones_sb = const.tile([1, P], F32, tag="ones")
            nc.gpsimd.memset(ones_sb[:], 1.0)
            kyT = persist.tile([P, KT, M], F32, tag="kyT")

            def proj_T(xt_sb, w_sb, out_sb, engines):
                """out[d, i] = sum_k W.T[k,d] * xT[k,i] (QT/KT layout)."""
                for ot in range(KT):
                    ps = psA.tile([P, M], F32, tag="ps512")
                    for kt in range(KT):
                        nc.tensor.matmul(
                            ps[:],
                            w_sb[:, kt, ot * P : (ot + 1) * P],
                            xt_sb[:, kt, :],
                            start=(kt == 0),
                            stop=(kt == KT - 1),
                        )
                    eng = engines[ot % len(engines)]
                    if eng == "v":
                        nc.vector.tensor_copy(out_sb[:, ot, :], ps[:])
                    else:
                        nc.scalar.copy(out_sb[:, ot, :], ps[:])

            def proj_V(xt_sb, w_sb, out_sb):
                """V[j, d] = sum_k xT[k,j] * W.T[k,d] (row layout)."""
                for jt in range(NT):
                    ps = psB.tile([P, D], F32, tag="ps256")
                    for kt in range(KT):
                        nc.tensor.matmul(
                            ps[:],
                            xt_sb[:, kt, jt * P : (jt + 1) * P],
                            w_sb[:, kt, :],
                            start=(kt == 0),
                            stop=(kt == KT - 1),
                        )
                    if jt % 2 == 0:
                        nc.vector.tensor_copy(out_sb[:, jt, :], ps[:])
                    else:
                        nc.scalar.copy(out_sb[:, jt, :], ps[:])

            def scores_softmax(
                qT_sb, kT_sb, colf_row, rowcol_base, e_sb, rrs_tiles, s_dram, mask_eng
            ):
                """Masked scores (optionally written) + E=exp(s-max), 1/rowsum."""
                for it in range(NT):
                    ps = psA.tile([P, M], F32, tag="ps512")
                    for kt in range(KT):
                        nc.tensor.matmul(
                            ps[:],
                            qT_sb[:, kt, it * P : (it + 1) * P],
                            kT_sb[:, kt, :],
                            start=(kt == 0),
                            stop=False,
                        )
                    nc.tensor.matmul(
                        ps[:],
                        ones_sb[:],
                        colf_sb[:, colf_row * M : (colf_row + 1) * M],
                        start=False,
                        stop=True,
                    )
                    ci = rowcol_base + it
                    rm = rowm_sb[:, ci : ci + 1]
                    ra = rowa_sb[:, ci : ci + 1]
                    s_sb = sout.tile([P, M], F32, tag="s_out")
                    if mask_eng == "v":
                        nc.vector.tensor_scalar(s_sb[:], ps[:], rm, ra, ALU.mult, ALU.add)
                    else:
                        nc.scalar.activation(s_sb[:], ps[:], AF.Identity, bias=ra, scale=rm)
                    if s_dram is not None:
                        nc.sync.dma_start(s_dram[it * P : (it + 1) * P, :], s_sb[:])
                    nmx = small.tile([P, 1], F32, tag="nmx")
                    nc.vector.tensor_reduce(
                        nmx[:], s_sb[:], axis=mybir.AxisListType.X, op=ALU.max, negate=True
                    )
                    rs = small.tile([P, 1], F32, tag="rs")
                    nc.scalar.activation(
                        e_sb[:, it, :], s_sb[:], AF.Exp, bias=nmx[:], scale=1.0,
                        accum_out=rs[:],
                    )
                    rrs = small.tile([P, 1], F32, tag="rrs")
                    nc.vector.reciprocal(rrs[:], rs[:])
                    rrs_tiles.append(rrs)

            def ctx_out(e_sb, v_sb, rrs_tiles, c_dram):
                """ctx = softmax(s) @ V via PE-transposed E."""
                et_sb = work.tile([P, NT, M], F32, tag="et")
                for it in range(NT):
                    for jt in range(NT):
                        pst = psC.tile([P, P], F32, tag="ps128")
                        nc.tensor.transpose(
                            pst[:], e_sb[:, it, jt * P : (jt + 1) * P], ident[:]
                        )
                        if (it + jt) % 2 == 0:
                            nc.vector.tensor_copy(
                                et_sb[:, jt, it * P : (it + 1) * P], pst[:]
                            )
                        else:
                            nc.scalar.copy(et_sb[:, jt, it * P : (it + 1) * P], pst[:])
                for it in range(NT):
                    psc = psB.tile([P, D], F32, tag="ps256")
                    for jt in range(NT):
                        nc.tensor.matmul(
                            psc[:],
                            et_sb[:, jt, it * P : (it + 1) * P],
                            v_sb[:, jt, :],
                            start=(jt == 0),
                            stop=(jt == NT - 1),
                        )
                    c_sb = sout.tile([P, D], F32, tag="c_out")
                    nc.vector.tensor_scalar(
                        c_sb[:], psc[:], rrs_tiles[it][:], None, ALU.mult
                    )
                    nc.sync.dma_start(c_dram[it * P : (it + 1) * P, :], c_sb[:])

            # ---------- y batch first (KyT stays resident) ----------
            yt_sb = work.tile([P, KT, M], F32, tag="xt")
            nc.sync.dma_start(yt_sb[:], yT.rearrange("k p m -> p k m"))
            qyT = work.tile([P, KT, M], F32, tag="qt")
            proj_T(yt_sb, wq_sb, qyT, ("s", "v"))
            proj_T(yt_sb, wk_sb, kyT, ("v", "s"))
            vy = work.tile([P, NT, D], F32, tag="v")
            proj_V(yt_sb, wv_sb, vy)
            e_y = work.tile([P, NT, M], F32, tag="e")
            rrs_y = []
            scores_softmax(qyT, kyT, NEG, NEG * NT, e_y, rrs_y, scores_y[:], "v")
            ctx_out(e_y, vy, rrs_y, ctx_y[:])

            # ---------- x batches ----------
            for b in range(NEG):
                xt_sb = work.tile([P, KT, M], F32, tag="xt")
                nc.sync.dma_start(xt_sb[:], xT[b].rearrange("k p m -> p k m"))
                qxT = work.tile([P, KT, M], F32, tag="qt")
                proj_T(xt_sb, wq_sb, qxT, ("s", "v"))
                kxT = work.tile([P, KT, M], F32, tag="kt")
                proj_T(xt_sb, wk_sb, kxT, ("v", "s"))
                vx = work.tile([P, NT, D], F32, tag="v")
                proj_V(xt_sb, wv_sb, vx)

                # x-x attention
                e_x = work.tile([P, NT, M], F32, tag="e")
                rrs_x = []
                scores_softmax(qxT, kxT, b, b * NT, e_x, rrs_x, scores_x[b], "v")
                ctx_out(e_x, vx, rrs_x, ctx_x[b])

                # x-y probs (no ctx, no scores output)
                e_xy = work.tile([P, NT, M], F32, tag="exy")
                rrs_xy = []
                scores_softmax(qxT, kyT, NEG, b * NT, e_xy, rrs_xy, None, "s")
                for it in range(NT):
                    p_sb = sout.tile([P, M], F32, tag="p_out")
                    nc.gpsimd.tensor_scalar(
                        p_sb[:], e_xy[:, it, :], rrs_xy[it][:], None, ALU.mult
                    )
                    nc.sync.dma_start(probs_xy[b, it * P : (it + 1) * P, :], p_sb[:])

    nc.compile()
    return nc


def _get_nc():
    if "nc" not in _CACHE:
        _CACHE["nc"] = build_kernel()
    return _CACHE["nc"]


def make_in_maps(x, y, mask_x, mask_y, Wq, Wk, Wv):
    x = np.ascontiguousarray(np.asarray(x, dtype=np.float32))
    y = np.ascontiguousarray(np.asarray(y, dtype=np.float32))
    mask_x = np.asarray(mask_x).astype(bool)
    mask_y = np.asarray(mask_y).astype(bool)
    Wq = np.asarray(Wq, dtype=np.float32)
    Wk = np.asarray(Wk, dtype=np.float32)
    Wv = np.asarray(Wv, dtype=np.float32)

    xTh = np.ascontiguousarray(x.transpose(0, 2, 1)).reshape(B, KT, P, M)
    yTh = np.ascontiguousarray(y.transpose(0, 2, 1)).reshape(BS, KT, P, M)
    wqT = np.ascontiguousarray(Wq.T).reshape(KT, P, D)
    wkT = np.ascontiguousarray(Wk.T).reshape(KT, P, D)
    wvT = np.ascontiguousarray(Wv.T).reshape(KT, P, D)
    mx_f = mask_x.astype(np.float32)
    my_f = mask_y.astype(np.float32)

    in_maps = []
    for s in range(BS):
        masks = [mx_f[s * NEG + n] for n in range(NEG)] + [my_f[s]]
        rowm = np.empty((P, NBATCH * NT), np.float32)
        for bi, mv in enumerate(masks):
            rowm[:, bi * NT : (bi + 1) * NT] = mv.reshape(NT, P).T
        rowa = (rowm - 1.0) * 1e10
        colf = ((np.stack(masks) - 1.0) * np.float32(1e10)).reshape(1, NBATCH * M)
        in_maps.append(
            {
                "xT": np.ascontiguousarray(xTh[s * NEG : (s + 1) * NEG]),
                "yT": np.ascontiguousarray(yTh[s]),
                "wqT": wqT,
                "wkT": wkT,
                "wvT": wvT,
                "rowm": np.ascontiguousarray(rowm),
                "rowa": np.ascontiguousarray(rowa),
                "colf": np.ascontiguousarray(colf.astype(np.float32)),
            }
        )

    return in_maps


def kernel(x, y, mask_x, mask_y, Wq, Wk, Wv):
    global LAST_RESULT
    mask_x = np.asarray(mask_x).astype(bool)
    mask_y = np.asarray(mask_y).astype(bool)
    in_maps = make_in_maps(x, y, mask_x, mask_y, Wq, Wk, Wv)
    res = run_bass_kernel_spmd(_get_nc(), in_maps, core_ids=list(range(BS)))
    LAST_RESULT = res
    results = res.results

    ctx_x = np.concatenate([r["ctx_x"] for r in results], axis=0)
    ctx_y = np.stack([r["ctx_y"] for r in results], axis=0)
    scores_x = np.concatenate([r["scores_x"] for r in results], axis=0)
    scores_y = np.stack([r["scores_y"] for r in results], axis=0)
    probs_xy = np.stack([r["probs_xy"] for r in results], axis=1)

    mask_xy = (
        mask_x.reshape(BS, NEG, M, 1).swapaxes(0, 1) & mask_y[:, None, :][None]
    )
    y_len = np.broadcast_to(
        mask_y.sum(axis=1).astype(np.int32)[None, :], (NEG, BS)
    ).copy()

    return (ctx_x, ctx_y, scores_x, scores_y, probs_xy, mask_xy, y_len)
